# revision 32
# baseline (speedup 1.0000x reference)
"""Cosine-similarity clustering layer (retrieval kNN) on 8 Trainium2 cores.

Computes sim = ((x/|x|) @ (c/|c|).T + 1) / 2 for x [64,512,1024], c [256,1024].

Strategy: data-parallel over the 32768 flattened rows of x (4096 rows per
core), cluster centers replicated. The host hands each core its x shard
TRANSPOSED (xt [1024, 4096] fp32, contraction dim outermost) so the device
never transposes x. Per core:
  - ScalarE act-func table (Square/Sqrt/Copy live in one set) is preloaded
    via a dummy Sqrt at t=0, hidden under the first DMAs
  - centers: fp32 norms on ScalarE (Square + accum), scale+cast to fp16
    split across ScalarE/DVE, transposed to cnT [d_part, chunk, k] on the
    PE (idle that early) in half-tile PSUM pieces
  - x streams in 4 m-blocks of [128d, 8chunk, 1024m] fp16 (4 MiB fp32 read
    each, fp32->fp16 cast in flight on the SWDGE queue; first block split
    into quarters so the PE can start at ~2.7us)
  - GEMM per 128-row m-tile: 8 accumulating fp16 matmuls [128d x 128m] @
    [128d x 256k] into PSUM psS (fp32), FWL-friendly (128-col weights)
  - row norms come from the SAME PE data: Gram matmuls lhsT=rhs=x-tile
    into a per-tile PSUM psG [128,128]; diag(psG) = |x_row|^2. Extracted
    with DVE identity-mask multiply + row-reduce, then sqrt(4*ss)=2|x| on
    ScalarE + DVE reciprocal -> rnh = 0.5/|x|. The Gram needs no centers,
    so the scheduler fills the entire centers-prep latency with Gram work.
  - epilogue folds the norm and (s+1)/2 affine into one op per m-tile,
    alternating ScalarE (activation Copy: psS*rnh + 0.5) and DVE
    (tensor_scalar); 2-tile batched stores on SP. The last block runs
    grams/extracts first so the post-GEMM tail is just epilogue + a
    single-tile store.
PE does 512 matmuls + 16 transposes (~43.5us busy at fp16) and runs
gapless from +3.7us in CoreSim (52.0us vs 72.0us for the previous kernel).

Hardware traps baked into this design (CoreSim accepts both, TRN2 dies):
  - tensor_tensor_reduce hard-crashes the exec unit (NRT_EXEC_UNIT_
    UNRECOVERABLE) -> use tensor_tensor + tensor_reduce
  - float16 ExternalInput DRAM tensors return garbage through the
    bass2jax/PJRT path -> ship fp32 and cast in the DMA
"""

import sys

import numpy as np

for _p in ("/opt/trn_rl_repo",):
    if _p not in sys.path:
        sys.path.insert(0, _p)

N_CORES = 8
B, S, D = 64, 512, 1024
K = 256                      # n_clusters
ROWS = (B * S) // N_CORES    # 4096 rows per core
P = 128
DCH = D // P                 # 8 contraction chunks
KT = K // P                  # 2 center tiles
MB = 1024                    # m columns per load block
NBLK = ROWS // MB            # 4 blocks
TPB = MB // P                # 4 m-tiles per block

_cache = {}


def _kvar():
    import os

    return set(v for v in os.environ.get("KVAR", "").split(",") if v)


def build_module():
    KV = _kvar()
    import concourse.bacc as bacc
    import concourse.mybir as mybir
    import concourse.tile as tile
    from concourse.masks import make_identity

    f32 = mybir.dt.float32
    f16 = mybir.dt.float16
    Act = mybir.ActivationFunctionType
    Alu = mybir.AluOpType

    nc = bacc.Bacc("TRN2", target_bir_lowering=False, debug=False)
    xt = nc.dram_tensor("xt", [D, ROWS], f32, kind="ExternalInput")
    c = nc.dram_tensor("c", [K, D], f32, kind="ExternalInput")
    out = nc.dram_tensor("out", [ROWS, K], f32, kind="ExternalOutput")

    with tile.TileContext(nc) as tc:
        with (
            tc.tile_pool(name="const", bufs=1) as cpool,
            tc.tile_pool(name="xload", bufs=4) as xpool,
            tc.tile_pool(name="norms", bufs=10) as npool,
            tc.tile_pool(name="mask", bufs=4) as mpool,
            tc.tile_pool(name="outp", bufs=3) as opool,
            tc.tile_pool(name="psum_s", bufs=4, space="PSUM") as pspool,
            tc.tile_pool(name="psum_g", bufs=3, space="PSUM") as pgpool,
            tc.tile_pool(name="psum_ct", bufs=1, space="PSUM") as ctpool,
        ):
            ident = cpool.tile([P, P], f16, name="ident")
            make_identity(nc, ident[:])

            # Preload the Sqrt act-func set (contains Square/Sqrt/Copy/
            # Identity) at t=0, hidden under the first DMAs; otherwise the
            # 1.3us table switch lands mid-centers-chain.
            warm = cpool.tile([P, 1], f32, name="warm")
            nc.scalar.activation(warm[:], ident[:, 0:1], Act.Sqrt)

            # ---- centers: fp32 norms, scale+cast to fp16, transpose ----
            # c loads on the SP queue (x stream owns Pool); the norm chain
            # is split across Act/DVE; transposes happen on the PE below.
            cnT = cpool.tile([P, DCH, K], f16, name="cnT")
            css = cpool.tile([P, KT], f32, name="css")
            cf_tiles = []
            for i in range(KT):
                cf = cpool.tile([P, D], f32, name=f"cf{i}")
                nc.sync.dma_start(cf[:], c[i * P : (i + 1) * P, :])
                csq = cpool.tile([P, D], f32, name="csq")
                nc.scalar.activation(
                    csq[:], cf[:], Act.Square, accum_out=css[:, i : i + 1]
                )
                cf_tiles.append(cf)
            # rc = 1/|c| (norms ~32 for randn rows; eps clamp unreachable)
            cnorm = cpool.tile([P, KT], f32, name="cnorm")
            rc = cpool.tile([P, KT], f32, name="rc")
            nc.scalar.activation(cnorm[:], css[:], Act.Sqrt)
            nc.vector.reciprocal(rc[:], cnorm[:])
            for i in range(KT):
                cb = cpool.tile([P, D], f16, name=f"cb{i}")
                if i == 0:
                    nc.scalar.activation(
                        cb[:], cf_tiles[i][:], Act.Copy, scale=rc[:, i : i + 1]
                    )
                else:
                    nc.vector.tensor_scalar_mul(
                        cb[:], cf_tiles[i][:], rc[:, i : i + 1]
                    )
                # transpose on the PE (idle this early; XBAR transposes kept
                # getting queued behind unrelated DMA completions):
                # cnT[p, o, i*128+m] = cb[m, o*128+p]
                for h in range(2):
                    hD = DCH // 2
                    psCT = ctpool.tile([P, hD, P], f16, name="psCT")
                    for jj in range(hD):
                        j = h * hD + jj
                        nc.tensor.transpose(
                            psCT[:, jj, :], cb[:, j * P : (j + 1) * P], ident[:]
                        )
                    dst = cnT[:, h * hD : (h + 1) * hD, i * P : (i + 1) * P]
                    if i == 0:
                        nc.vector.tensor_copy(dst, psCT[:])
                    else:
                        nc.scalar.activation(dst, psCT[:], Act.Copy)

            # ---- x m-blocks: [128 d, 8 chunks, 1024 m] fp16, cast in flight ----
            xb0_shared = None
            for g in range(NBLK):
                if "noload" in KV and g > 0:
                    xb = xb0_shared
                elif "noload" in KV:
                    xb = cpool.tile([P, DCH, MB], f16, name="xb0s")
                    xb0_shared = xb
                else:
                    xb = xpool.tile([P, DCH, MB], f16, name="xb")
                if "noload" in KV and g > 0:
                    pass
                elif g == 0:
                    # split the first load so PE can start earlier
                    q = MB // 4
                    for h in range(4):
                        nc.gpsimd.dma_start(
                            xb[:, :, h * q : (h + 1) * q],
                            xt[:, h * q : (h + 1) * q].rearrange(
                                "(j p) m -> p j m", p=P
                            ),
                        )
                else:
                    nc.gpsimd.dma_start(
                        xb[:],
                        xt[:, g * MB : (g + 1) * MB].rearrange(
                            "(j p) m -> p j m", p=P
                        ),
                    )
                obat = opool.tile([P, TPB, K], f32, name="obat")
                psG_tiles = {}
                psS_tiles = []

                def gram(i):
                    psG = pgpool.tile([P, P], f32, name="psG")
                    psG_tiles[i] = psG
                    for j in range(DCH):
                        lhs = xb[:, j, i * P : (i + 1) * P]
                        nc.tensor.matmul(
                            psG[:],
                            lhs,
                            lhs,
                            start=(j == 0),
                            stop=(j == DCH - 1),
                        )

                def gemm(i):
                    psS = psS_tiles[i]
                    for j in range(DCH):
                        lhs = xb[:, j, i * P : (i + 1) * P]
                        nc.tensor.matmul(
                            psS[:],
                            lhs,
                            cnT[:, j, :],
                            start=(j == 0),
                            stop=(j == DCH - 1),
                        )

                def extract(i):
                    # diag(psG4[:,i,:]) = |x_row|^2 for m-tile i
                    msk = mpool.tile([P, P], f16, name="msk")
                    ss = npool.tile([P, 1], f32, name="ss")
                    nc.vector.tensor_tensor(
                        msk[:], psG_tiles[i][:], ident[:], Alu.mult
                    )
                    nc.vector.tensor_reduce(
                        ss[:], msk[:], mybir.AxisListType.X, Alu.add
                    )
                    # rnh = 0.5/|x_row|: sqrt(4*ss) = 2|x|, then reciprocal
                    rnh = npool.tile([P, 1], f32, name="rnh")
                    nc.scalar.activation(rnh[:], ss[:], Act.Sqrt, scale=4.0)
                    nc.vector.reciprocal(rnh[:], rnh[:])
                    return rnh

                def epilogue(i, rnh):
                    # out = psS * (0.5/|x|) + 0.5, engines alternated
                    if i % 2 == 0:
                        nc.scalar.activation(
                            obat[:, i, :],
                            psS_tiles[i][:],
                            Act.Copy,
                            bias=0.5,
                            scale=rnh if isinstance(rnh, float) else rnh[:],
                        )
                    else:
                        nc.vector.tensor_scalar(
                            obat[:, i, :],
                            psS_tiles[i][:],
                            rnh if isinstance(rnh, float) else rnh[:],
                            0.5,
                            Alu.mult,
                            Alu.add,
                        )

                def store(i):
                    # store tiles i-1, i
                    rr = g * MB + (i - 1) * P
                    nc.sync.dma_start(
                        out[rr : rr + 2 * P, :].rearrange(
                            "(n p) k -> p n k", p=P
                        ),
                        obat[:, i - 1 : i + 1, :],
                    )

                for i in range(TPB):
                    psS_tiles.append(pspool.tile([P, K], f32, name="psS"))
                nogram = "nogram" in KV
                if g == 0 and not nogram:
                    # grams first: they only need x, not the centers chain
                    for i in range(TPB):
                        gram(i)
                    for i in range(TPB):
                        gemm(i)
                        rnh = extract(i)
                        epilogue(i, rnh)
                        if i % 2 == 1:
                            store(i)
                elif nogram:
                    for i in range(TPB):
                        gemm(i)
                        epilogue(i, 0.015625)
                        if i % 2 == 1:
                            store(i)
                elif g == NBLK - 1:
                    # last block: norms first so the post-GEMM tail is just
                    # epilogue + a short final store
                    rnhs = []
                    for i in range(TPB):
                        gram(i)
                        rnhs.append(extract(i))
                    for i in range(TPB):
                        gemm(i)
                        epilogue(i, rnhs[i])
                        if i % 2 == 1 and i < TPB - 2:
                            store(i)
                        elif i >= TPB - 2:
                            rr = g * MB + i * P
                            nc.sync.dma_start(
                                out[rr : rr + P, :].rearrange(
                                    "(n p) k -> p n k", p=P
                                ),
                                obat[:, i : i + 1, :],
                            )
                else:
                    for i in range(TPB):
                        gemm(i)
                        gram(i)
                        rnh = extract(i)
                        epilogue(i, rnh)
                        if i % 2 == 1:
                            store(i)
    nc.compile()
    return nc


def get_module():
    if "nc" not in _cache:
        _cache["nc"] = build_module()
    return _cache["nc"]


OUT_NAMES = ["out"]


def shard_inputs(x2d, c):
    maps = []
    for s in np.split(x2d, N_CORES, axis=0):
        maps.append(
            {
                "xt": np.ascontiguousarray(s.T),
                "c": np.ascontiguousarray(c),
            }
        )
    return maps


def unshard_core0(outs):
    return outs["out"]


def unshard_full(outs):
    # outs: name -> [n_cores, *core_shape]
    return outs["out"].reshape(-1, K)


def kernel(x, cluster_centers):
    from concourse.bass_utils import run_bass_kernel_spmd

    x = np.ascontiguousarray(np.asarray(x, dtype=np.float32))
    c = np.ascontiguousarray(np.asarray(cluster_centers, dtype=np.float32))
    b, s, d = x.shape
    xf = x.reshape(-1, d)
    nc = get_module()
    in_maps = shard_inputs(xf, c)
    res = run_bass_kernel_spmd(nc, in_maps, list(range(N_CORES)))
    outs = [np.asarray(res.results[i]["out"]) for i in range(N_CORES)]
    return np.concatenate(outs, axis=0).reshape(b, s, K)
